# revision 37
# baseline (speedup 1.0000x reference)
"""Causal self-attention (B=2, T=2048, D=1024, H=16) on 8 NeuronCores.

Sharding: heads across cores (2 heads/core). x is pre-transposed on the
host (DMA-transpose is descriptor-gen bound at ~100GB/s and starved the
PE for the first 100us). Each core:
  - loads x^T with plain contiguous DMA, finest stripes first so qkv
    tile 0 unlocks after ~4us,
  - computes qT/kT/vT for its heads (W_qkv column shard), bf16 matmuls,
  - flash-style causal attention with scores transposed [k, q] so that
    att@v needs no transposes (ones-column on v gives softmax sums);
    per-head score tiles, diagonal blocks narrowed to live columns,
  - computes a partial y @ W_proj with its 128 rows of W_proj; the host
    sums the 8 cores' partials.
Program order interleaves qkv(tt) with attention supertiles one-to-one
so the PE always has dense filler while ACT crunches exp:
  qkv(0) attn(b0,0) qkv(1) attn(b0,1) ... qkv(7) attn(b1,3) proj(7)
PSUM banks: st(scores+qkv)=3, yt(av accum)=3, aux(pp+vn)=2.
"""
import numpy as np
import ml_dtypes
from contextlib import ExitStack

import concourse.bass as bass
import concourse.tile as tile
from concourse import bacc, mybir
from concourse.bass_utils import run_bass_kernel_spmd
from concourse.masks import make_identity

B, T, D, H, HD = 2, 2048, 1024, 16, 64
NCORES = 8
BT = B * T                    # 4096
DQ = 128                      # head dims per core (2 heads x 64)
TT = 512                      # t-tile for the qkv phase
NTT = BT // TT                # 8
NDC = D // 128                # 8 contraction chunks
NQS = T // 512                # 4 q-supertiles per batch
SCALE = 1.0 / np.sqrt(HD)     # 0.125
ROWS = BT // NCORES           # 512 output rows per core
f32 = mybir.dt.float32
bf16 = mybir.dt.bfloat16
BF16NP = ml_dtypes.bfloat16


def _pin_act_table(arch):
    """Make Exp and Ln resolve to the ONE table set containing both
    (natural_log_exp_and_others), so the kernel needs a single
    ACT_TABLE_LOAD instead of thrashing 1.3us reloads between the score
    exps and the ln-based reciprocal. Only the *contents* used for set
    selection are edited; set indices (act_func_set_id) are untouched.
    """
    from concourse.hw_specs import get_activation_tables

    tabs = get_activation_tables(arch)  # functools.cache'd dict
    exp = mybir.ActivationFunctionType.Exp
    ln = mybir.ActivationFunctionType.Ln
    for name, fns in tabs.items():
        if name != "natural_log_exp_and_others":
            fns.discard(exp)
            fns.discard(ln)


def build_module(debug=False):
    nc = bacc.Bacc("TRN2", target_bir_lowering=False, debug=False, num_devices=NCORES)
    _pin_act_table(nc.m.arch)
    xt_d = nc.dram_tensor("xt", [D, BT], bf16, kind="ExternalInput").ap()
    # wqkv pre-arranged on host to the exact SBUF layout [qkv, d%128, d//128, dq]
    wqkv_d = nc.dram_tensor("wqkv", [3, 128, NDC, DQ], bf16, kind="ExternalInput").ap()
    wp_d = nc.dram_tensor("wp", [128, D], bf16, kind="ExternalInput").ap()
    out_d = nc.dram_tensor("out", [BT, D], bf16, kind="ExternalOutput").ap()
    if debug:
        dbg = {
            "qT": nc.dram_tensor("dbg_qT", [128, BT], bf16, kind="ExternalOutput").ap(),
            "kT": nc.dram_tensor("dbg_kT", [128, BT], bf16, kind="ExternalOutput").ap(),
            "vext": nc.dram_tensor("dbg_vext", [128, BT // 128, 2, HD + 1], bf16, kind="ExternalOutput").ap(),
            "yTn": nc.dram_tensor("dbg_yTn", [128, BT], bf16, kind="ExternalOutput").ap(),
            "xt0": nc.dram_tensor("dbg_xt0", [128, BT], bf16, kind="ExternalOutput").ap(),
        }

    with tile.TileContext(nc) as tc, ExitStack() as ctx:
        const = ctx.enter_context(tc.tile_pool(name="const", bufs=1))
        wpool = ctx.enter_context(tc.tile_pool(name="w", bufs=1))
        big = ctx.enter_context(tc.tile_pool(name="big", bufs=1))
        vtp = ctx.enter_context(tc.tile_pool(name="vt", bufs=2))
        expp = ctx.enter_context(tc.tile_pool(name="exp", bufs=8))
        smal = ctx.enter_context(tc.tile_pool(name="small", bufs=4))
        outp = ctx.enter_context(tc.tile_pool(name="outsb", bufs=2))
        psum = ctx.enter_context(tc.tile_pool(name="ps", bufs=1, space="PSUM"))

        ident = const.tile([128, 128], bf16)
        make_identity(nc, ident[:])
        ones64 = const.tile([1, HD], bf16)
        nc.gpsimd.memset(ones64[:], 1.0)

        # PE warmup: dummy transposes while the x/w DMAs land, so the HAM
        # clock gate is already released (2.4GHz) when qkv starts.
        for i in range(24):
            wps = psum.tile([128, 128], bf16, tag="st", bufs=2, name=f"warm{i}")
            nc.tensor.transpose(wps[:], ident[:], ident[:])

        # Load order on the sync ring: wqkv-q, x stripe 0, wqkv-k/v, then
        # the remaining x stripes — so the first qkv matmul group (q of
        # tile 0) can start as early as possible. All contiguous
        # host-prearranged loads (x is transposed on the host).
        wqkv_sb = wpool.tile([128, 3, NDC, DQ], bf16)
        wp_sb = wpool.tile([128, D], bf16)
        xtf = []
        for dc in range(NDC):
            xt = big.tile([128, BT], bf16, tag=f"xtf{dc}", name=f"xtf{dc}")
            xtf.append(xt)
        nc.sync.dma_start(wqkv_sb[:, 0], wqkv_d[0])
        for dc in range(NDC):
            nc.sync.dma_start(xtf[dc][:, 0:512], xt_d[128 * dc : 128 * (dc + 1), 0:512])
        nc.sync.dma_start(wqkv_sb[:, 1], wqkv_d[1])
        nc.sync.dma_start(wqkv_sb[:, 2], wqkv_d[2])
        nc.gpsimd.dma_start(wp_sb[:], wp_d[:])
        for r0, r1 in ((512, 1024), (1024, 2048), (2048, 3072), (3072, 4096)):
            for dc in range(NDC):
                nc.sync.dma_start(
                    xtf[dc][:, r0:r1],
                    xt_d[128 * dc : 128 * (dc + 1), r0:r1],
                )

        qT_sb = big.tile([128, BT], bf16)    # [2 heads x 64, b*T+t]
        kT_sb = big.tile([128, BT], bf16)
        # v natural + ones column: [k%128, k//128, head, hd+1]
        vext_sb = big.tile([128, BT // 128, 2, HD + 1], bf16)
        nc.gpsimd.memset(vext_sb[:, :, :, HD : HD + 1], 1.0)
        yTn_sb = big.tile([128, BT], bf16)   # normalized y^T

        def qkv_tile(tt):
            # q -> k -> v sequentially
            t0 = TT * tt
            vts = vtp.tile([128, TT], bf16, tag="vts", name=f"vts{tt}")
            for qi in range(3):
                ps = psum.tile([128, TT], f32, tag="st", bufs=2, name=f"qkv{tt}_{qi}")
                for dc in range(NDC):
                    nc.tensor.matmul(
                        ps[:],
                        wqkv_sb[:, qi, dc, :],
                        xtf[dc][:, t0 : t0 + TT],
                        start=(dc == 0),
                        stop=(dc == NDC - 1),
                    )
                if qi == 0:
                    nc.vector.tensor_copy(qT_sb[:, t0 : t0 + TT], ps[:])
                elif qi == 1:
                    nc.vector.tensor_copy(kT_sb[:, t0 : t0 + TT], ps[:])
                else:
                    nc.vector.tensor_copy(vts[:], ps[:])
            return vts

        def v_transposes(tt, vts):
            # [128 dq, 512 t] -> vext natural chunks, via one psum tile
            vn = psum.tile([128, 4, 128], bf16, tag="aux", bufs=2, name=f"vn{tt}")
            for s in range(4):
                nc.tensor.transpose(vn[:, s, :], vts[:, 128 * s : 128 * (s + 1)], ident[:])
            for h in range(2):
                nc.vector.tensor_copy(
                    vext_sb[:, 4 * tt : 4 * tt + 4, h, 0:HD], vn[:, :, 64 * h : 64 * h + 64]
                )

        def emit_scores(b, qs, kt, qg):
            """Scores+exp for one (128 k)x(512 q) block, both heads in one
            2-bank psum tile -> a single exp and a single affine per block.
            Diagonal blocks are narrowed to their live columns [c0, 512)."""
            d = kt - 4 * qs  # >= 0 on the diagonal supertile
            c0 = 128 * d if d > 0 else 0
            kg = 2048 * b + 128 * kt
            st = psum.tile([128, 2, 512], f32, tag="st", bufs=2, name=f"st{b}_{qs}_{kt}")
            for h in range(2):
                nc.tensor.matmul(
                    st[:, h, c0:512],
                    kT_sb[64 * h : 64 * h + 64, kg : kg + 128],
                    qT_sb[64 * h : 64 * h + 64, qg + c0 : qg + 512],
                    start=True,
                    stop=True,
                )
            est = expp.tile([128, 2, 512], bf16, tag="est", name=f"est{b}_{qs}_{kt}")
            nc.scalar.activation(
                est[:, :, c0:512], st[:, :, c0:512], mybir.ActivationFunctionType.Exp,
                scale=SCALE,
            )
            if d >= 0:  # diagonal: zero where k > q inside the 128-wide window
                nc.gpsimd.affine_select(
                    out=est[:, :, c0 : c0 + 128],
                    in_=est[:, :, c0 : c0 + 128],
                    pattern=[[0, 2], [1, 128]],
                    compare_op=mybir.AluOpType.is_ge,
                    fill=0.0,
                    base=0,
                    channel_multiplier=-1,
                )
            return est, c0

        def proj_partial(j, final=False):
            # partial projection (this core's 128 head-dims) for output rows
            # [512j, 512j+512); host sums the 8 cores' partials. The final
            # chunk runs after all attention: spread over the freed score
            # and yt psum slots (4 in flight) and drain on both DVE and ACT
            # to shorten the tail.
            for ts in range(4):
                c0 = 512 * j + 128 * ts
                for half in range(2):
                    u = 2 * ts + half
                    if final:
                        tag, bufs = ("st", 2) if u % 2 == 0 else ("yt", 2)
                    else:
                        tag, bufs = "aux", 2
                    pp = psum.tile(
                        [128, 512], f32, tag=tag, bufs=bufs, name=f"pp{j}_{ts}_{half}"
                    )
                    nc.tensor.matmul(
                        pp[:],
                        yTn_sb[:, c0 : c0 + 128],
                        wp_sb[:, 512 * half : 512 * (half + 1)],
                        start=True,
                        stop=True,
                    )
                    otag = "osbf" if final else "osb"
                    osb = outp.tile([128, 512], bf16, tag=otag, bufs=4 if final else 2,
                                    name=f"osb{j}{ts}{half}")
                    if final and u % 2 == 1:
                        nc.scalar.copy(osb[:], pp[:])
                    else:
                        nc.vector.tensor_copy(osb[:], pp[:])
                    nc.sync.dma_start(
                        out_d[c0 : c0 + 128, 512 * half : 512 * (half + 1)], osb[:]
                    )

        def attention_block(b, qs, pre_emit=None):
            """One q-supertile of causal attention, software-pipelined two
            score blocks ahead so the PE never waits on the exp chain."""
            qg = 2048 * b + 512 * qs
            nkt = 4 * qs + 4
            yt0 = psum.tile([HD + 1, 512], f32, tag="yt", bufs=2, name=f"yt0_{b}_{qs}")
            yt1 = psum.tile([HD + 1, 512], f32, tag="yt", bufs=2, name=f"yt1_{b}_{qs}")
            yt = [yt0, yt1]
            ests = {0: emit_scores(b, qs, 0, qg)}
            if pre_emit is not None:
                pre_emit()  # v-transposes / proj chunk: PE filler for the pipe head
            if nkt > 1:
                ests[1] = emit_scores(b, qs, 1, qg)
            for kt in range(nkt):
                est, c0 = ests.pop(kt)
                if kt + 2 < nkt:
                    ests[kt + 2] = emit_scores(b, qs, kt + 2, qg)
                kchunk = (2048 * b + 128 * kt) // 128
                for h in range(2):
                    nc.tensor.matmul(
                        yt[h][:, c0:512],
                        vext_sb[:, kchunk, h, :],
                        est[:, h, c0:512],
                        start=(kt == 0),
                        stop=(kt == nkt - 1),
                    )
            # softmax normalization: sums live in row HD of each yt psum.
            # 1/s = exp(-ln(s)) on ACT (DVE reciprocal costs 3.3us/call on
            # HW; exp+ln live in one ACT table set so this is two fast ops).
            # The broadcast runs on the PE (ones-vector matmul) — gpsimd
            # partition_broadcast needs a different Q7 library than
            # affine_select, and the per-supertile library-reload code
            # fetch queues behind bulk DMA (12us stalls observed).
            # Emission is DEFERRED past the next qkv tile so the PE FIFO
            # never waits on the ACT chain.
            # Part 1 (inline, DVE/ACT only): copy y out unnormalized — this
            # frees the yt psum slots — and compute 1/sums into recb.
            parts = []
            for h in range(2):
                dst = yTn_sb[64 * h : 64 * h + 64, qg : qg + 512]
                nc.vector.tensor_copy(dst, yt[h][0:HD, :])
                lg = smal.tile([1, 512], f32, tag="lg", name=f"lg_{b}_{qs}_{h}")
                nc.scalar.activation(
                    lg[:], yt[h][HD : HD + 1, :], mybir.ActivationFunctionType.Ln
                )
                recb = smal.tile([1, 512], bf16, tag="recb", name=f"recb_{b}_{qs}_{h}")
                nc.scalar.activation(
                    recb[:], lg[:], mybir.ActivationFunctionType.Exp, scale=-1.0
                )
                parts.append((dst, recb))

            # Part 2 (deferred past the next qkv tile, so the PE FIFO never
            # waits on the ACT chain): PE-broadcast of 1/s and the scale.
            def norm():
                for h, (dst, recb) in enumerate(parts):
                    bcp = psum.tile([HD, 512], f32, tag="aux", bufs=2, name=f"bc_{b}_{qs}_{h}")
                    nc.tensor.matmul(bcp[:], ones64[:], recb[:], start=True, stop=True)
                    nc.vector.tensor_mul(dst, dst, bcp[:])

            return 4 * b + qs, norm

        # ---- interleave qkv tiles with attention supertiles 1:1 ----
        last_j = None
        pending_norm = None
        for b in range(B):
            for qs in range(NQS):
                tt = 4 * b + qs
                vts = qkv_tile(tt)
                if pending_norm is not None:
                    pending_norm()

                def pre(t=tt, v=vts, pj=last_j):
                    v_transposes(t, v)
                    if pj is not None:
                        proj_partial(pj)

                last_j, pending_norm = attention_block(b, qs, pre_emit=pre)
        pending_norm()
        proj_partial(last_j, final=True)

        if debug:
            nc.sync.dma_start(dbg["qT"][:], qT_sb[:])
            nc.sync.dma_start(dbg["kT"][:], kT_sb[:])
            nc.sync.dma_start(dbg["vext"][:], vext_sb[:])
            nc.sync.dma_start(dbg["yTn"][:], yTn_sb[:])
            nc.sync.dma_start(dbg["xt0"][:], xtf[1][:])

    nc.compile()
    return nc


_NC_CACHE = None


def _get_module():
    global _NC_CACHE
    if _NC_CACHE is None:
        _NC_CACHE = build_module()
    return _NC_CACHE


def make_in_maps(x, W_qkv, W_proj):
    x2 = np.ascontiguousarray(
        np.asarray(x, dtype=np.float32).reshape(BT, D).astype(BF16NP).T
    )
    wq = np.asarray(W_qkv, dtype=np.float32)
    wp_full = np.asarray(W_proj, dtype=np.float32)
    in_maps = []
    for c in range(NCORES):
        wp = np.ascontiguousarray(wp_full[128 * c : 128 * (c + 1), :].astype(BF16NP))
        wl = np.stack(
            [
                wq[:, 128 * c : 128 * (c + 1)],
                wq[:, D + 128 * c : D + 128 * (c + 1)],
                wq[:, 2 * D + 128 * c : 2 * D + 128 * (c + 1)],
            ],
            axis=1,
        )  # [D, 3, DQ]
        # [d, q, m] -> [q, d%128, d//128, m]: the exact SBUF layout
        wl = np.ascontiguousarray(
            wl.reshape(NDC, 128, 3, DQ).transpose(2, 1, 0, 3).astype(BF16NP)
        )
        in_maps.append({"xt": x2, "wqkv": wl, "wp": wp})
    return in_maps


def run(x, W_qkv, W_proj, **spmd_kwargs):
    nc = _get_module()
    in_maps = make_in_maps(x, W_qkv, W_proj)
    res = run_bass_kernel_spmd(nc, in_maps, list(range(NCORES)), **spmd_kwargs)
    out = np.zeros((BT, D), dtype=np.float32)
    for c in range(NCORES):
        out += res.results[c]["out"].astype(np.float32)
    return out.reshape(B, T, D), res


def kernel(x, W_qkv, W_proj):
    out, _ = run(x, W_qkv, W_proj)
    return out
